# revision 32
# baseline (speedup 1.0000x reference)
"""Trainium2 Bass kernel for nn_MultiHeadAttention_63986422775834.

Computation (see harness reference):
    q = x @ Wq + bq; k = x @ Wk + bk; v = x @ Wv + bv          # [N, D]
    group rows by 8: scores[b,h,g] = q[8b+h] . k[8b+g] / sqrt(D)
    w = softmax(scores, axis=-1);  out[8b+h] = sum_g w[b,h,g] * v[8b+g]

Key algebraic reduction: softmax is over the group axis g, so terms of
q.k^T that are constant along g cancel.  With M = Wq Wk^T and
v2 = Wk bq:
    softmax(q k^T) == softmax(t' x^T)  where t' = x M + 1 v2^T
(x Wq bk^T and bq bk^T are g-constant; bq.(x Wk) folds into the
per-partition bias v2 of the t' GEMM).  M is row-count independent, so
its 2048^3 cost is computed ONCE, sharded over the 8 cores (each core
computes a 256-row strip from host-pretransposed WqT/WkT) and shared
via an HBM AllGather.  Per-core tensor work drops from 3 big GEMMs to
2 + 1/8th of M.

Sharding: data-parallel over rows across 8 NeuronCores (2048 rows each;
row groups of 8 never cross a shard boundary).  Host pre-transposes and
casts x / weights to bf16 (no on-device transposes needed) and upcasts
the bf16 output back to fp32.

Measured HW notes driving the structure:
 - One engine queue issues DMAs serially at ~230-250 GB/s; emission
   order IS the schedule.  WkT chunks go first (the M phase consumes
   them at exactly the DMA rate), then xT / Wv; dependency-free filler
   matmuls bridge the DMA-bound gap between the M and V phases.
 - Issuing a collective drops the PE duty-cycle cap from 15/16 to
   13/16 for the remainder of the kernel (HAM type-31), so the AG is
   issued as early as possible and everything it gates is minimized.
 - S accumulates directly in PSUM (mask preloaded, all matmuls
   accumulate); softmax runs per-block inside the t phase.
 - The tail streams V chunks 2,3 and interleaves the resident-half O
   matmuls between chains as PE filler while output DMA drains.

Per-core phase order:
  M phase   : M[256-strip, :] = WqTs^T @ WkT   (65K PE cycles)
  AllGather : 1 MB -> 8 MB bf16 in DRAM (hidden under the V phase)
  V phase   : v = x Wv + bv for d_out 0:1024, kept resident in SBUF
  t phase   : t' = x M + v2 streamed in 512-col chunks; S += t' x^T
              into PSUM per 128-row tile; per-block softmax
  tail      : O = w V; resident-half O interleaved with streamed V
              chunks 2,3; bf16 output DMA overlapped
"""

import sys

sys.path.insert(0, "/opt/trn_rl_repo")

import numpy as np
import ml_dtypes

import concourse.mybir as mybir
import concourse.tile as tile
from concourse.tile_rust import add_dep_helper
from concourse import bacc
from concourse.bass_utils import run_bass_kernel_spmd

# problem shape (hardcoded per contract)
N_FULL = 16384
D = 2048
H = 8
N_CORES = 8
R = N_FULL // N_CORES  # rows per core = 2048
P = 128
KO = D // P  # 16 k-subtiles along the contraction dim
MS = D // N_CORES  # 256-row M strip per core
SCALE = 1.0 / np.sqrt(np.float32(D))

BF16 = mybir.dt.bfloat16
F32 = mybir.dt.float32

BLOCKS = [(0, 512), (512, 512), (1024, 512), (1536, 512)]  # row blocks
CW = 512  # chunk width for all streamed weight/M chunks
NC_CHUNKS = D // CW  # 4
V_RES = 2  # V chunks computed early and kept resident (d_out 0:1024)
NT = R // P  # 16 row tiles per core


def build_program():
    nc = bacc.Bacc("TRN2", target_bir_lowering=False, debug=False, num_devices=N_CORES)

    xsTb = nc.dram_tensor("xsTb", [P, 4, KO, 512], BF16, kind="ExternalInput")
    WqTsb = nc.dram_tensor("WqTsb", [P, KO, MS], BF16, kind="ExternalInput")
    WkTb = nc.dram_tensor("WkTb", [P, NC_CHUNKS, KO, CW], BF16, kind="ExternalInput")
    Wvb = nc.dram_tensor("Wvb", [P, NC_CHUNKS, KO, CW], BF16, kind="ExternalInput")
    v2t = nc.dram_tensor("v2t", [P, KO], F32, kind="ExternalInput")
    bvr = nc.dram_tensor("bvr", [P, D], BF16, kind="ExternalInput")
    maskt = nc.dram_tensor("maskt", [P, P], F32, kind="ExternalInput")
    ident = nc.dram_tensor("ident", [P, P], BF16, kind="ExternalInput")
    out = nc.dram_tensor("out", [R, D], BF16, kind="ExternalOutput")

    msh = nc.dram_tensor("msh", [MS, D], BF16)  # this core's M strip
    # rows 0:D = gathered M; rows D:D+8 = barrier area written by a second
    # tiny AllGather (collectives run in order on the CC engine, so its
    # completion implies every peer's M strip has landed; readers of gath
    # are ordered after it by DRAM dependency tracking)
    gath = nc.dram_tensor("gath", [D + 8, D], BF16, addr_space="Shared")
    bar_in = nc.dram_tensor("bar_in", [1, D], BF16)

    # partition-sliced view of the gathered M: m[p, ko, n] = M[ko*128 + p, n]
    m_ap = gath[0:D, :].rearrange("(ko p) n -> p ko n", p=P)

    with tile.TileContext(nc) as tc:
        with (
            tc.tile_pool(name="const", bufs=1) as const,
            tc.tile_pool(name="xT", bufs=1) as xT_pool,
            tc.tile_pool(name="vres", bufs=1) as vres,
            tc.tile_pool(name="wqts", bufs=1) as wqtsp,
            tc.tile_pool(name="wchunk", bufs=4) as wchunk,
            tc.tile_pool(name="qk", bufs=8) as qkp,
            tc.tile_pool(name="sacc", bufs=1) as sacc,
            tc.tile_pool(name="sacc_s", bufs=1) as sacc_s,
            tc.tile_pool(name="soft", bufs=2) as soft,
            tc.tile_pool(name="vpool", bufs=3) as vpool,
            tc.tile_pool(name="obuf", bufs=3) as obuf,
            tc.tile_pool(name="ps_big", bufs=4, space="PSUM") as ps_big,
            tc.tile_pool(name="ps_s", bufs=3, space="PSUM") as ps_s,
            tc.tile_pool(name="ps_t", bufs=1, space="PSUM") as ps_t,
        ):
            # --- constants (only ident is needed early, for the warmup;
            # the rest load after the startup-critical weight/x DMAs) ---
            ident_sb = const.tile([P, P], BF16)
            nc.sync.dma_start(ident_sb, ident[:])

            # HAM warm-up: dependency-free matmuls keep the PE busy/full-rate
            # through the DMA-bound startup window.
            for _ in range(100):
                wps = ps_big.tile([P, CW], F32, tag="ps_big", name="wps")[:, :P]
                nc.tensor.matmul(wps, lhsT=ident_sb, rhs=ident_sb, start=True, stop=True)

            def load_m_chunk(c):
                dst = wchunk.tile([P, KO, CW], BF16, tag="w", name="wchunk")
                d = nc.sync.dma_start(dst, m_ap[:, :, c * CW : (c + 1) * CW])
                # explicit edge: M reads only after the barrier AllGather
                # (slice-aware DRAM tracking doesn't connect them)
                add_dep_helper(
                    getattr(d, "ins", d), getattr(ag_bar, "ins", ag_bar),
                    sync=True, reason="gath reads wait for AG barrier",
                )
                return dst

            def load_w_chunk(t, c):  # tile-layout weight chunk: one 16KB seg/p
                dst = wchunk.tile([P, KO, CW], BF16, tag="w", name="wchunk")
                nc.sync.dma_start(dst, t[:, c, :, :])
                return dst

            # --- hoisted DMAs (one serial in-order queue; emission order =
            # schedule).  Interleaved so M-phase (wkt) and V-phase (xb/wv)
            # inputs arrive just in time and the PE alternates between the
            # two phases without idling.
            wqts_sb = wqtsp.tile([P, KO, MS], BF16)
            nc.sync.dma_start(wqts_sb, WqTsb[:])
            wkt_tiles = {c: load_w_chunk(WkTb, c) for c in range(2)}

            # x^T row-block tiles: xT[bi][p, kt, r] = x[bi*512 + r, kt*128+p]
            xT = []

            def load_x_block(bi):
                t = xT_pool.tile([P, KO, 512], BF16, tag=f"xT{bi}", name="xTb")
                nc.sync.dma_start(t, xsTb[:, bi, :, :])
                xT.append(t)

            load_x_block(0)
            wv_tiles = {0: load_w_chunk(Wvb, 0)}
            wkt_tiles[2] = load_w_chunk(WkTb, 2)
            load_x_block(1)
            wkt_tiles[3] = load_w_chunk(WkTb, 3)
            load_x_block(2)
            load_x_block(3)

            # late constants + S accumulator init (mask) — needed from the
            # t phase onward; DVE init precedes all S adds in queue order
            mask_sb = const.tile([P, P], F32)
            nc.sync.dma_start(mask_sb, maskt[:])
            v2_sb = const.tile([P, KO], F32)
            nc.sync.dma_start(v2_sb, v2t[:])
            bv_sb = const.tile([P, D], BF16)
            nc.sync.dma_start(bv_sb, bvr[:])
            S_all = sacc_s.tile([P, NT, P], F32, name="S_all")
            for i in range(NT):
                nc.vector.tensor_copy(S_all[:, i, :], mask_sb)

            # resident first-half V: V_all[p, i, d] = v[i*128 + p, d], d < 1024
            V_all = vres.tile([P, KO, V_RES * CW], BF16, name="V_all")

            def xt_slice(rs):  # [P, KO, 128] view of row tile rs
                return xT[rs // 4][:, :, (rs % 4) * P : (rs % 4 + 1) * P]

            def emit_m_chunk(bc):
                wkt_sb = wkt_tiles.pop(bc)
                for ah in range(MS // P):
                    psm = ps_big.tile([P, CW], F32, tag="ps_big", name="psm")
                    for os_ in range(KO):
                        nc.tensor.matmul(
                            psm,
                            lhsT=wqts_sb[:, os_, ah * P : (ah + 1) * P],
                            rhs=wkt_sb[:, os_, :],
                            start=(os_ == 0),
                            stop=(os_ == KO - 1),
                        )
                    msb = obuf.tile([P, CW], BF16, tag="msh", name="msb")
                    nc.vector.tensor_copy(msb, psm)
                    nc.sync.dma_start(
                        msh[ah * P : (ah + 1) * P, bc * CW : (bc + 1) * CW], msb
                    )

            def emit_v_chain(c, rs, wv_sb):
                psv = ps_big.tile([P, CW], F32, tag="ps_big", name="psv")
                xs = xt_slice(rs)
                for kt in range(KO):
                    nc.tensor.matmul(
                        psv,
                        lhsT=xs[:, kt, :],
                        rhs=wv_sb[:, kt, :],
                        start=(kt == 0),
                        stop=(kt == KO - 1),
                    )
                return psv

            # --- interleaved M phase + V chunk 0 ---
            emit_m_chunk(0)
            emit_m_chunk(1)
            # filler: the V start waits on wv0's DMA (~4.4us); keep the PE
            # busy so HAM doesn't demote the clock across the gap
            for _ in range(42):
                wps = ps_big.tile([P, CW], F32, tag="ps_big", name="wps")[:, :P]
                nc.tensor.matmul(wps, lhsT=ident_sb, rhs=ident_sb, start=True, stop=True)
            wv0_sb = wv_tiles.pop(0)
            for rs in range(4):
                psv = emit_v_chain(0, rs, wv0_sb)
                nc.vector.tensor_copy(V_all[:, rs, 0:CW], psv)
            emit_m_chunk(2)
            for rs in range(4, 8):
                psv = emit_v_chain(0, rs, wv0_sb)
                nc.vector.tensor_copy(V_all[:, rs, 0:CW], psv)
            emit_m_chunk(3)

            # --- AllGather the M strips (completes under the V phase) ---
            nc.gpsimd.collective_compute(
                "AllGather",
                mybir.AluOpType.bypass,
                replica_groups=[list(range(N_CORES))],
                ins=[msh[:]],
                outs=[gath[0:D, :]],
            )
            # chain: bar_in <- gath forces AG2 after AG1; AG2's completion
            # implies every peer's strip landed (in-order CC execution)
            nc.sync.dma_start(bar_in[:], gath[0:1, :])
            ag_bar = nc.gpsimd.collective_compute(
                "AllGather",
                mybir.AluOpType.bypass,
                replica_groups=[list(range(N_CORES))],
                ins=[bar_in[:]],
                outs=[gath[D : D + 8, :]],
            )

            wv_tiles[1] = load_w_chunk(Wvb, 1)
            for rs in range(8, NT):
                psv = emit_v_chain(0, rs, wv0_sb)
                nc.vector.tensor_copy(V_all[:, rs, 0:CW], psv)
            wv1_sb = wv_tiles.pop(1)
            for rs in range(NT):
                psv = emit_v_chain(1, rs, wv1_sb)
                nc.vector.tensor_copy(V_all[:, rs, CW : 2 * CW], psv)

            # --- t phase: t' = x M + v2; S += t' x^T (PSUM chain -> SBUF add) ---
            wT_all = sacc.tile([P, NT, P], BF16, name="wT_all")

            def emit_s(bi, c, tts):
                row0, nrows = BLOCKS[bi]
                for sub in range(nrows // P):
                    i = row0 // P + sub
                    pss = ps_s.tile([P, P], F32, tag="pss", name="pss")
                    for jj in range(len(tts)):
                        nc.tensor.matmul(
                            pss,
                            lhsT=tts[jj][:, sub * P : (sub + 1) * P],
                            rhs=xT[bi][
                                :, c * (CW // P) + jj, sub * P : (sub + 1) * P
                            ],
                            start=(jj == 0),
                            stop=(jj == len(tts) - 1),
                        )
                    nc.vector.tensor_add(S_all[:, i, :], S_all[:, i, :], pss)

            def emit_softmax(i):
                e = soft.tile([P, P], F32, tag="e")
                ssum = soft.tile([P, 1], F32, tag="ssum")
                nc.scalar.activation(
                    e, S_all[:, i, :], mybir.ActivationFunctionType.Exp,
                    scale=float(SCALE), accum_out=ssum,
                )
                rcp = soft.tile([P, 1], F32, tag="rcp")
                nc.vector.reciprocal(rcp, ssum)
                wsb = soft.tile([P, P], BF16, tag="wsb")
                nc.vector.tensor_scalar_mul(wsb, e, rcp)
                pst = ps_t.tile([P, P], BF16, tag="tr")
                nc.tensor.transpose(pst, wsb, ident_sb)
                nc.vector.tensor_copy(wT_all[:, i, :], pst)

            m_tiles = {0: load_m_chunk(0)}
            pending_s = None  # (bi, c, tts)
            for c in range(NC_CHUNKS):
                if c + 1 < NC_CHUNKS and (c + 1) not in m_tiles:
                    m_tiles[c + 1] = load_m_chunk(c + 1)
                if c == 1:
                    wv_tiles[2] = load_w_chunk(Wvb, 2)
                elif c == 2:
                    wv_tiles[3] = load_w_chunk(Wvb, 3)
                m_sb = m_tiles.pop(c)
                for bi, (row0, nrows) in enumerate(BLOCKS):
                    tts = []
                    for jj in range(CW // P):
                        j = c * (CW // P) + jj
                        psq = ps_big.tile([P, CW], F32, tag="ps_big", name="psq")
                        for kt in range(KO):
                            nc.tensor.matmul(
                                psq,
                                lhsT=m_sb[:, kt, jj * P : (jj + 1) * P],
                                rhs=xT[bi][:, kt, :],
                                start=(kt == 0),
                                stop=(kt == KO - 1),
                            )
                        tt = qkp.tile([P, CW], BF16, tag="qk", name="tt")
                        nc.scalar.activation(
                            tt, psq, mybir.ActivationFunctionType.Identity,
                            bias=v2_sb[:, j : j + 1],
                        )
                        tts.append(tt)
                    if pending_s is not None:
                        emit_s(*pending_s)
                        if pending_s[1] == NC_CHUNKS - 1:  # S final for that block
                            for sub in range(4):
                                emit_softmax(pending_s[0] * 4 + sub)
                    pending_s = (bi, c, tts)
            if pending_s is not None:
                emit_s(*pending_s)
                for sub in range(4):
                    emit_softmax(pending_s[0] * 4 + sub)
                pending_s = None

            # --- tail: O = w V + bv ---
            def emit_o(v_src, i, col0, width):
                pso = ps_big.tile([P, CW], F32, tag="ps_big", name="pso")[:, :width]
                nc.tensor.matmul(
                    pso, lhsT=wT_all[:, i, :], rhs=v_src, start=True, stop=True
                )
                o_sb = obuf.tile([P, CW], BF16, tag="o", name="o_sb")[:, :width]
                nc.vector.tensor_add(o_sb, pso, bv_sb[:, col0 : col0 + width])
                r0 = i * P
                nc.sync.dma_start(out[r0 : r0 + P, col0 : col0 + width], o_sb)

            # streamed second half; the 32 resident-half O matmuls are
            # interleaved between chains as PE filler
            resident_os = [(i, cc) for i in range(NT) for cc in range(V_RES)]
            pending_o = None
            for c in range(V_RES, NC_CHUNKS):
                wv_sb = wv_tiles.pop(c)
                for rs in range(NT):
                    psv = emit_v_chain(c, rs, wv_sb)
                    v_sb = vpool.tile([P, CW], BF16, tag="v", name="v_sb")
                    nc.vector.tensor_copy(v_sb, psv)
                    if resident_os:
                        i, cc = resident_os.pop(0)
                        emit_o(V_all[:, i, cc * CW : (cc + 1) * CW], i, cc * CW, CW)
                    if pending_o is not None:
                        emit_o(*pending_o)
                    pending_o = (v_sb, rs, c * CW, CW)
            while resident_os:
                i, cc = resident_os.pop(0)
                emit_o(V_all[:, i, cc * CW : (cc + 1) * CW], i, cc * CW, CW)
            if pending_o is not None:
                emit_o(*pending_o)
                pending_o = None


    nc.compile()
    return nc


_CACHED = {}


def host_constants():
    mask = np.full((P, P), -1e9, dtype=np.float32)
    for g in range(P // H):
        mask[g * H : (g + 1) * H, g * H : (g + 1) * H] = 0.0
    identity = np.eye(P, dtype=ml_dtypes.bfloat16)
    return mask, identity


def prepare_in_maps(x, Wq, bq, Wk, bk, Wv, bv):
    x = np.asarray(x, dtype=np.float32)
    Wq = np.asarray(Wq, dtype=np.float32)
    Wk = np.asarray(Wk, dtype=np.float32)
    Wv = np.asarray(Wv, dtype=np.float32)
    bq = np.asarray(bq, dtype=np.float32)
    bv = np.asarray(bv, dtype=np.float32)

    mask, identity = host_constants()

    def tile_layout(a):  # [2048, W] -> [128, W/512, 16, 512], 16KB segs
        w = a.shape[1]
        return np.ascontiguousarray(
            a.reshape(KO, P, w // CW, CW).transpose(1, 2, 0, 3)
        )

    xT_bf = x.T.astype(ml_dtypes.bfloat16)  # [D, N]
    WqT_bf = Wq.T.astype(ml_dtypes.bfloat16)
    WkTb = tile_layout(Wk.T.astype(ml_dtypes.bfloat16))
    Wvb = tile_layout(Wv.astype(ml_dtypes.bfloat16))
    v2 = (Wk @ bq).astype(np.float32)
    v2t = np.ascontiguousarray(v2.reshape(KO, P).T)
    bvr = np.ascontiguousarray(
        np.broadcast_to(bv.astype(ml_dtypes.bfloat16), (P, D))
    )

    in_maps = []
    for i in range(N_CORES):
        in_maps.append(
            {
                "xsTb": tile_layout(xT_bf[:, i * R : (i + 1) * R]),
                "WqTsb": np.ascontiguousarray(
                    WqT_bf[:, i * MS : (i + 1) * MS]
                    .reshape(KO, P, MS)
                    .transpose(1, 0, 2)
                ),
                "WkTb": WkTb,
                "Wvb": Wvb,
                "v2t": v2t,
                "bvr": bvr,
                "maskt": mask,
                "ident": identity,
            }
        )
    return in_maps


def assemble_output(res):
    return np.concatenate(
        [res.results[i]["out"].astype(np.float32) for i in range(N_CORES)], axis=0
    )


def kernel(x, Wq, bq, Wk, bk, Wv, bv):
    if "nc" not in _CACHED:
        _CACHED["nc"] = build_program()
    nc = _CACHED["nc"]
    in_maps = prepare_in_maps(x, Wq, bq, Wk, bk, Wv, bv)
    res = run_bass_kernel_spmd(nc, in_maps, list(range(N_CORES)))
    return assemble_output(res)


# revision 33
# speedup vs baseline: 1.0093x; 1.0093x over previous
"""Trainium2 Bass kernel for nn_MultiHeadAttention_63986422775834.

Computation (see harness reference):
    q = x @ Wq + bq; k = x @ Wk + bk; v = x @ Wv + bv          # [N, D]
    group rows by 8: scores[b,h,g] = q[8b+h] . k[8b+g] / sqrt(D)
    w = softmax(scores, axis=-1);  out[8b+h] = sum_g w[b,h,g] * v[8b+g]

Key algebraic reduction: softmax is over the group axis g, so terms of
q.k^T that are constant along g cancel.  With M = Wq Wk^T and
v2 = Wk bq:
    softmax(q k^T) == softmax(t' x^T)  where t' = x M + 1 v2^T
(x Wq bk^T and bq bk^T are g-constant; bq.(x Wk) folds into the
per-partition bias v2 of the t' GEMM).  M is row-count independent, so
its 2048^3 cost is computed ONCE, sharded over the 8 cores (each core
computes a 256-row strip from host-pretransposed WqT/WkT) and shared
via an HBM AllGather.  Per-core tensor work drops from 3 big GEMMs to
2 + 1/8th of M.

Sharding: data-parallel over rows across 8 NeuronCores (2048 rows each;
row groups of 8 never cross a shard boundary).  Host pre-transposes and
casts x / weights to bf16 (no on-device transposes needed) and upcasts
the bf16 output back to fp32.

Measured HW notes driving the structure:
 - One engine queue issues DMAs serially at ~230-250 GB/s; emission
   order IS the schedule.  WkT chunks go first (the M phase consumes
   them at exactly the DMA rate), then xT / Wv; dependency-free filler
   matmuls bridge the DMA-bound gap between the M and V phases.
 - Issuing a collective drops the PE duty-cycle cap from 15/16 to
   13/16 for the remainder of the kernel (HAM type-31), so the AG is
   issued as early as possible and everything it gates is minimized.
 - S accumulates directly in PSUM (mask preloaded, all matmuls
   accumulate); softmax runs per-block inside the t phase.
 - The tail streams V chunks 2,3 and interleaves the resident-half O
   matmuls between chains as PE filler while output DMA drains.

Per-core phase order:
  M phase   : M[256-strip, :] = WqTs^T @ WkT   (65K PE cycles)
  AllGather : 1 MB -> 8 MB bf16 in DRAM (hidden under the V phase)
  V phase   : v = x Wv + bv for d_out 0:1024, kept resident in SBUF
  t phase   : t' = x M + v2 streamed in 512-col chunks; S += t' x^T
              into PSUM per 128-row tile; per-block softmax
  tail      : O = w V; resident-half O interleaved with streamed V
              chunks 2,3; bf16 output DMA overlapped
"""

import sys

sys.path.insert(0, "/opt/trn_rl_repo")

import numpy as np
import ml_dtypes

import concourse.mybir as mybir
import concourse.tile as tile
from concourse.tile_rust import add_dep_helper
from concourse import bacc
from concourse.bass_utils import run_bass_kernel_spmd

# problem shape (hardcoded per contract)
N_FULL = 16384
D = 2048
H = 8
N_CORES = 8
R = N_FULL // N_CORES  # rows per core = 2048
P = 128
KO = D // P  # 16 k-subtiles along the contraction dim
MS = D // N_CORES  # 256-row M strip per core
SCALE = 1.0 / np.sqrt(np.float32(D))

BF16 = mybir.dt.bfloat16
F32 = mybir.dt.float32

BLOCKS = [(0, 512), (512, 512), (1024, 512), (1536, 512)]  # row blocks
CW = 512  # chunk width for all streamed weight/M chunks
NC_CHUNKS = D // CW  # 4
V_RES = 2  # V chunks computed early and kept resident (d_out 0:1024)
NT = R // P  # 16 row tiles per core


def build_program():
    nc = bacc.Bacc("TRN2", target_bir_lowering=False, debug=False, num_devices=N_CORES)

    xsTb = nc.dram_tensor("xsTb", [P, 4, KO, 512], BF16, kind="ExternalInput")
    WqTsb = nc.dram_tensor("WqTsb", [P, KO, MS], BF16, kind="ExternalInput")
    WkTb = nc.dram_tensor("WkTb", [P, NC_CHUNKS, KO, CW], BF16, kind="ExternalInput")
    Wvb = nc.dram_tensor("Wvb", [P, NC_CHUNKS, KO, CW], BF16, kind="ExternalInput")
    v2t = nc.dram_tensor("v2t", [P, KO], F32, kind="ExternalInput")
    bvr = nc.dram_tensor("bvr", [P, D], BF16, kind="ExternalInput")
    maskt = nc.dram_tensor("maskt", [P, P], F32, kind="ExternalInput")
    ident = nc.dram_tensor("ident", [P, P], BF16, kind="ExternalInput")
    out = nc.dram_tensor("out", [R, D], BF16, kind="ExternalOutput")

    msh = nc.dram_tensor("msh", [MS, D], BF16)  # this core's M strip
    # rows 0:D = gathered M; rows D:D+8 = barrier area written by a second
    # tiny AllGather (collectives run in order on the CC engine, so its
    # completion implies every peer's M strip has landed; readers of gath
    # are ordered after it by DRAM dependency tracking)
    gath = nc.dram_tensor("gath", [D + 8, D], BF16, addr_space="Shared")
    bar_in = nc.dram_tensor("bar_in", [1, D], BF16)

    # partition-sliced view of the gathered M: m[p, ko, n] = M[ko*128 + p, n]
    m_ap = gath[0:D, :].rearrange("(ko p) n -> p ko n", p=P)

    with tile.TileContext(nc) as tc:
        with (
            tc.tile_pool(name="const", bufs=1) as const,
            tc.tile_pool(name="xT", bufs=1) as xT_pool,
            tc.tile_pool(name="vres", bufs=1) as vres,
            tc.tile_pool(name="wqts", bufs=1) as wqtsp,
            tc.tile_pool(name="wchunk", bufs=4) as wchunk,
            tc.tile_pool(name="qk", bufs=8) as qkp,
            tc.tile_pool(name="sacc", bufs=1) as sacc,
            tc.tile_pool(name="sacc_s", bufs=1) as sacc_s,
            tc.tile_pool(name="soft", bufs=2) as soft,
            tc.tile_pool(name="vpool", bufs=3) as vpool,
            tc.tile_pool(name="obuf", bufs=3) as obuf,
            tc.tile_pool(name="ps_big", bufs=4, space="PSUM") as ps_big,
            tc.tile_pool(name="ps_s", bufs=3, space="PSUM") as ps_s,
            tc.tile_pool(name="ps_t", bufs=1, space="PSUM") as ps_t,
        ):
            # --- constants (only ident is needed early, for the warmup;
            # the rest load after the startup-critical weight/x DMAs) ---
            ident_sb = const.tile([P, P], BF16)
            nc.sync.dma_start(ident_sb, ident[:])

            # HAM warm-up: dependency-free matmuls keep the PE busy/full-rate
            # through the DMA-bound startup window.
            for _ in range(100):
                wps = ps_big.tile([P, CW], F32, tag="ps_big", name="wps")[:, :P]
                nc.tensor.matmul(wps, lhsT=ident_sb, rhs=ident_sb, start=True, stop=True)

            def load_m_chunk(c):
                dst = wchunk.tile([P, KO, CW], BF16, tag="w", name="wchunk")
                d = nc.sync.dma_start(dst, m_ap[:, :, c * CW : (c + 1) * CW])
                # explicit edge: M reads only after the barrier AllGather
                # (slice-aware DRAM tracking doesn't connect them)
                add_dep_helper(
                    getattr(d, "ins", d), getattr(ag_bar, "ins", ag_bar),
                    sync=True, reason="gath reads wait for AG barrier",
                )
                return dst

            def load_w_chunk(t, c):  # tile-layout weight chunk: one 16KB seg/p
                dst = wchunk.tile([P, KO, CW], BF16, tag="w", name="wchunk")
                nc.sync.dma_start(dst, t[:, c, :, :])
                return dst

            # --- hoisted DMAs (one serial in-order queue; emission order =
            # schedule).  Interleaved so M-phase (wkt) and V-phase (xb/wv)
            # inputs arrive just in time and the PE alternates between the
            # two phases without idling.
            wqts_sb = wqtsp.tile([P, KO, MS], BF16)
            nc.sync.dma_start(wqts_sb, WqTsb[:])
            wkt_tiles = {c: load_w_chunk(WkTb, c) for c in range(2)}

            # x^T row-block tiles: xT[bi][p, kt, r] = x[bi*512 + r, kt*128+p]
            xT = []

            def load_x_block(bi):
                t = xT_pool.tile([P, KO, 512], BF16, tag=f"xT{bi}", name="xTb")
                nc.sync.dma_start(t, xsTb[:, bi, :, :])
                xT.append(t)

            load_x_block(0)
            wv_tiles = {0: load_w_chunk(Wvb, 0)}
            wkt_tiles[2] = load_w_chunk(WkTb, 2)
            load_x_block(1)
            wkt_tiles[3] = load_w_chunk(WkTb, 3)
            load_x_block(2)
            load_x_block(3)

            # late constants + S accumulator init (mask) — needed from the
            # t phase onward; DVE init precedes all S adds in queue order
            mask_sb = const.tile([P, P], F32)
            nc.sync.dma_start(mask_sb, maskt[:])
            v2_sb = const.tile([P, KO], F32)
            nc.sync.dma_start(v2_sb, v2t[:])
            bv_sb = const.tile([P, D], BF16)
            nc.sync.dma_start(bv_sb, bvr[:])
            S_all = sacc_s.tile([P, NT, P], F32, name="S_all")
            for i in range(NT):
                nc.vector.tensor_copy(S_all[:, i, :], mask_sb)

            # resident first-half V: V_all[p, i, d] = v[i*128 + p, d], d < 1024
            V_all = vres.tile([P, KO, V_RES * CW], BF16, name="V_all")

            def xt_slice(rs):  # [P, KO, 128] view of row tile rs
                return xT[rs // 4][:, :, (rs % 4) * P : (rs % 4 + 1) * P]

            def emit_m_chunk(bc):
                wkt_sb = wkt_tiles.pop(bc)
                for ah in range(MS // P):
                    psm = ps_big.tile([P, CW], F32, tag="ps_big", name="psm")
                    for os_ in range(KO):
                        nc.tensor.matmul(
                            psm,
                            lhsT=wqts_sb[:, os_, ah * P : (ah + 1) * P],
                            rhs=wkt_sb[:, os_, :],
                            start=(os_ == 0),
                            stop=(os_ == KO - 1),
                        )
                    msb = obuf.tile([P, CW], BF16, tag="msh", name="msb")
                    nc.vector.tensor_copy(msb, psm)
                    nc.sync.dma_start(
                        msh[ah * P : (ah + 1) * P, bc * CW : (bc + 1) * CW], msb
                    )

            def emit_v_chain(c, rs, wv_sb):
                psv = ps_big.tile([P, CW], F32, tag="ps_big", name="psv")
                xs = xt_slice(rs)
                for kt in range(KO):
                    nc.tensor.matmul(
                        psv,
                        lhsT=xs[:, kt, :],
                        rhs=wv_sb[:, kt, :],
                        start=(kt == 0),
                        stop=(kt == KO - 1),
                    )
                return psv

            # --- interleaved M phase + V chunk 0 ---
            emit_m_chunk(0)
            emit_m_chunk(1)
            wv0_sb = wv_tiles.pop(0)
            for rs in range(4):
                psv = emit_v_chain(0, rs, wv0_sb)
                nc.vector.tensor_copy(V_all[:, rs, 0:CW], psv)
            emit_m_chunk(2)
            for rs in range(4, 8):
                psv = emit_v_chain(0, rs, wv0_sb)
                nc.vector.tensor_copy(V_all[:, rs, 0:CW], psv)
            emit_m_chunk(3)

            # --- AllGather the M strips (completes under the V phase) ---
            nc.gpsimd.collective_compute(
                "AllGather",
                mybir.AluOpType.bypass,
                replica_groups=[list(range(N_CORES))],
                ins=[msh[:]],
                outs=[gath[0:D, :]],
            )
            # chain: bar_in <- gath forces AG2 after AG1; AG2's completion
            # implies every peer's strip landed (in-order CC execution)
            nc.sync.dma_start(bar_in[:], gath[0:1, :])
            ag_bar = nc.gpsimd.collective_compute(
                "AllGather",
                mybir.AluOpType.bypass,
                replica_groups=[list(range(N_CORES))],
                ins=[bar_in[:]],
                outs=[gath[D : D + 8, :]],
            )

            wv_tiles[1] = load_w_chunk(Wvb, 1)
            for rs in range(8, NT):
                psv = emit_v_chain(0, rs, wv0_sb)
                nc.vector.tensor_copy(V_all[:, rs, 0:CW], psv)
            wv1_sb = wv_tiles.pop(1)
            for rs in range(NT):
                psv = emit_v_chain(1, rs, wv1_sb)
                nc.vector.tensor_copy(V_all[:, rs, CW : 2 * CW], psv)

            # --- t phase: t' = x M + v2; S += t' x^T (PSUM chain -> SBUF add) ---
            wT_all = sacc.tile([P, NT, P], BF16, name="wT_all")

            def emit_s(bi, c, tts):
                row0, nrows = BLOCKS[bi]
                for sub in range(nrows // P):
                    i = row0 // P + sub
                    pss = ps_s.tile([P, P], F32, tag="pss", name="pss")
                    for jj in range(len(tts)):
                        nc.tensor.matmul(
                            pss,
                            lhsT=tts[jj][:, sub * P : (sub + 1) * P],
                            rhs=xT[bi][
                                :, c * (CW // P) + jj, sub * P : (sub + 1) * P
                            ],
                            start=(jj == 0),
                            stop=(jj == len(tts) - 1),
                        )
                    nc.vector.tensor_add(S_all[:, i, :], S_all[:, i, :], pss)

            def emit_softmax(i):
                e = soft.tile([P, P], F32, tag="e")
                ssum = soft.tile([P, 1], F32, tag="ssum")
                nc.scalar.activation(
                    e, S_all[:, i, :], mybir.ActivationFunctionType.Exp,
                    scale=float(SCALE), accum_out=ssum,
                )
                rcp = soft.tile([P, 1], F32, tag="rcp")
                nc.vector.reciprocal(rcp, ssum)
                wsb = soft.tile([P, P], BF16, tag="wsb")
                nc.vector.tensor_scalar_mul(wsb, e, rcp)
                pst = ps_t.tile([P, P], BF16, tag="tr")
                nc.tensor.transpose(pst, wsb, ident_sb)
                nc.vector.tensor_copy(wT_all[:, i, :], pst)

            m_tiles = {0: load_m_chunk(0)}
            pending_s = None  # (bi, c, tts)
            for c in range(NC_CHUNKS):
                if c + 1 < NC_CHUNKS and (c + 1) not in m_tiles:
                    m_tiles[c + 1] = load_m_chunk(c + 1)
                if c == 1:
                    wv_tiles[2] = load_w_chunk(Wvb, 2)
                elif c == 2:
                    wv_tiles[3] = load_w_chunk(Wvb, 3)
                m_sb = m_tiles.pop(c)
                for bi, (row0, nrows) in enumerate(BLOCKS):
                    tts = []
                    for jj in range(CW // P):
                        j = c * (CW // P) + jj
                        psq = ps_big.tile([P, CW], F32, tag="ps_big", name="psq")
                        for kt in range(KO):
                            nc.tensor.matmul(
                                psq,
                                lhsT=m_sb[:, kt, jj * P : (jj + 1) * P],
                                rhs=xT[bi][:, kt, :],
                                start=(kt == 0),
                                stop=(kt == KO - 1),
                            )
                        tt = qkp.tile([P, CW], BF16, tag="qk", name="tt")
                        nc.scalar.activation(
                            tt, psq, mybir.ActivationFunctionType.Identity,
                            bias=v2_sb[:, j : j + 1],
                        )
                        tts.append(tt)
                    if pending_s is not None:
                        emit_s(*pending_s)
                        if pending_s[1] == NC_CHUNKS - 1:  # S final for that block
                            for sub in range(4):
                                emit_softmax(pending_s[0] * 4 + sub)
                    pending_s = (bi, c, tts)
            if pending_s is not None:
                emit_s(*pending_s)
                for sub in range(4):
                    emit_softmax(pending_s[0] * 4 + sub)
                pending_s = None

            # --- tail: O = w V + bv ---
            def emit_o(v_src, i, col0, width):
                pso = ps_big.tile([P, CW], F32, tag="ps_big", name="pso")[:, :width]
                nc.tensor.matmul(
                    pso, lhsT=wT_all[:, i, :], rhs=v_src, start=True, stop=True
                )
                o_sb = obuf.tile([P, CW], BF16, tag="o", name="o_sb")[:, :width]
                nc.vector.tensor_add(o_sb, pso, bv_sb[:, col0 : col0 + width])
                r0 = i * P
                nc.sync.dma_start(out[r0 : r0 + P, col0 : col0 + width], o_sb)

            # streamed second half; the 32 resident-half O matmuls are
            # interleaved between chains as PE filler
            resident_os = [(i, cc) for i in range(NT) for cc in range(V_RES)]
            pending_o = None
            for c in range(V_RES, NC_CHUNKS):
                wv_sb = wv_tiles.pop(c)
                for rs in range(NT):
                    psv = emit_v_chain(c, rs, wv_sb)
                    v_sb = vpool.tile([P, CW], BF16, tag="v", name="v_sb")
                    nc.vector.tensor_copy(v_sb, psv)
                    if resident_os:
                        i, cc = resident_os.pop(0)
                        emit_o(V_all[:, i, cc * CW : (cc + 1) * CW], i, cc * CW, CW)
                    if pending_o is not None:
                        emit_o(*pending_o)
                    pending_o = (v_sb, rs, c * CW, CW)
            while resident_os:
                i, cc = resident_os.pop(0)
                emit_o(V_all[:, i, cc * CW : (cc + 1) * CW], i, cc * CW, CW)
            if pending_o is not None:
                emit_o(*pending_o)
                pending_o = None


    nc.compile()
    return nc


_CACHED = {}


def host_constants():
    mask = np.full((P, P), -1e9, dtype=np.float32)
    for g in range(P // H):
        mask[g * H : (g + 1) * H, g * H : (g + 1) * H] = 0.0
    identity = np.eye(P, dtype=ml_dtypes.bfloat16)
    return mask, identity


def prepare_in_maps(x, Wq, bq, Wk, bk, Wv, bv):
    x = np.asarray(x, dtype=np.float32)
    Wq = np.asarray(Wq, dtype=np.float32)
    Wk = np.asarray(Wk, dtype=np.float32)
    Wv = np.asarray(Wv, dtype=np.float32)
    bq = np.asarray(bq, dtype=np.float32)
    bv = np.asarray(bv, dtype=np.float32)

    mask, identity = host_constants()

    def tile_layout(a):  # [2048, W] -> [128, W/512, 16, 512], 16KB segs
        w = a.shape[1]
        return np.ascontiguousarray(
            a.reshape(KO, P, w // CW, CW).transpose(1, 2, 0, 3)
        )

    xT_bf = x.T.astype(ml_dtypes.bfloat16)  # [D, N]
    WqT_bf = Wq.T.astype(ml_dtypes.bfloat16)
    WkTb = tile_layout(Wk.T.astype(ml_dtypes.bfloat16))
    Wvb = tile_layout(Wv.astype(ml_dtypes.bfloat16))
    v2 = (Wk @ bq).astype(np.float32)
    v2t = np.ascontiguousarray(v2.reshape(KO, P).T)
    bvr = np.ascontiguousarray(
        np.broadcast_to(bv.astype(ml_dtypes.bfloat16), (P, D))
    )

    in_maps = []
    for i in range(N_CORES):
        in_maps.append(
            {
                "xsTb": tile_layout(xT_bf[:, i * R : (i + 1) * R]),
                "WqTsb": np.ascontiguousarray(
                    WqT_bf[:, i * MS : (i + 1) * MS]
                    .reshape(KO, P, MS)
                    .transpose(1, 0, 2)
                ),
                "WkTb": WkTb,
                "Wvb": Wvb,
                "v2t": v2t,
                "bvr": bvr,
                "maskt": mask,
                "ident": identity,
            }
        )
    return in_maps


def assemble_output(res):
    return np.concatenate(
        [res.results[i]["out"].astype(np.float32) for i in range(N_CORES)], axis=0
    )


def kernel(x, Wq, bq, Wk, bk, Wv, bv):
    if "nc" not in _CACHED:
        _CACHED["nc"] = build_program()
    nc = _CACHED["nc"]
    in_maps = prepare_in_maps(x, Wq, bq, Wk, bk, Wv, bv)
    res = run_bass_kernel_spmd(nc, in_maps, list(range(N_CORES)))
    return assemble_output(res)
